# revision 21
# baseline (speedup 1.0000x reference)
"""Multi-head self-attention (B=2, T=4096, D=768, H=12) on 8 TRN2 NeuronCores.

Sharding: (batch, head)-parallel. Core c (0..7) handles batch b=c//4 and the
3 heads h0=(c%4)*3 .. h0+2.  Each core computes Q/K/V projections for its
heads, full softmax(QK^T/sqrt(d))V attention, and a partial output projection
through its 192 rows of Wo.  The host sums the 4 partials per batch and adds
the output bias bo.

Per-core pipeline (v5):
  Phase 1: Q/K projections into [d, t] layout; two t-tiles interleaved per
  weight chunk so LDWEIGHTS amortizes 2x; bias adds on ACT.  V in natural
  [t, d] layout with a trailing ones column (row 64 of the PV output
  accumulates the softmax denominator); V copies on ACT.
  Phase 2, per 512-wide q tile: 24 groups of 2 k-steps.  Each step: a
  row-tiled pair of score MMs (S^T[k,q] for 2 head-slots) into ONE 2-bank
  [128,1024] PSUM tile; exp split asymmetrically between ACT (exact exp,
  front XSPL cols on even steps / back XSPL on odd) and DVE (Schraudolph
  bitcast exp: i16 = round(23.083*s + 16250.5) = bf16 e^(s/8), ~3%
  pointwise) -- the DVE's effective throughput is ~half its op cost (pipe
  drain), so it gets the smaller share.  The previous group's 4 PV MMs run
  batched (fewer LDWEIGHTS transitions).  h0/h1 use groups 0-15; h2 uses
  groups 16-23 (Q/K duplicated onto both row halves, 2 k-tiles/step).
  Normalization: denominator row -> SBUF (ACT), fast reciprocal on [1,512]
  (DVE), K=1 broadcast matmul of the reciprocal, then one multiply (DVE).
  Wo accumulates in PSUM and is DMAed to HBM as f32 directly (no copy).
"""

import os
import numpy as np
import ml_dtypes

B, T, D = 2, 4096, 768
H, DH = 12, 64
NCORES = 8
HPC = 3            # heads per core
KC = D // 128      # 6 contraction chunks for projections
NT = T // 512      # 8 q tiles of 512
TT = T // 128      # 32 k tiles of 128

# Schraudolph constants: i16 = round(s * SCH_A + SCH_B); bits viewed as bf16
# give e^(s/8) with ~3.3% max pointwise error.
SCH_A = 128 / float(np.log(2)) * 0.125   # 23.08312065
SCH_B = 16256.0 - 5.5

BF16 = ml_dtypes.bfloat16

_CACHE = {}


def _trace(nc, tc, mybir, tens, iters=1):
    import concourse.bass as bass
    from contextlib import ExitStack

    ablate = os.environ.get("MHSA_ABLATE", "")
    XSPL = 512 + int(os.environ.get("MHSA_X", "192"))  # ACT's exp share

    f32 = mybir.dt.float32
    bf16 = mybir.dt.bfloat16
    i16 = mybir.dt.int16
    f32r = mybir.dt.float32r
    Exp = mybir.ActivationFunctionType.Exp
    Ident = mybir.ActivationFunctionType.Identity
    PSUM = bass.MemorySpace.PSUM
    Mult = mybir.AluOpType.mult
    Add = mybir.AluOpType.add

    with ExitStack() as ctx:
        persist = ctx.enter_context(tc.tile_pool(name="persist", bufs=1))

        # ---- persistent SBUF ----
        x_ch = [
            persist.tile([128, T], bf16, name=f"xc{kc}") for kc in range(KC)
        ]
        w_q = persist.tile([128, KC, HPC * DH], bf16)
        w_k = persist.tile([128, KC, HPC * DH], bf16)
        w_v = persist.tile([128, KC, HPC * DH], bf16)
        bq01 = persist.tile([128, 1], f32)
        bq2 = persist.tile([64, 1], f32)
        bk01 = persist.tile([128, 1], f32)
        bk2 = persist.tile([64, 1], f32)
        bv_sb = persist.tile([1, HPC * DH], bf16)
        ones1 = persist.tile([1, 128], bf16)     # K=1 lhsT for V bias MM
        ones65 = persist.tile([DH + 1, DH + 1], f32r)  # row 64: K=1 denom bcast lhsT
        q01 = persist.tile([128, T], bf16)       # h0 rows 0:64, h1 rows 64:128
        k01 = persist.tile([128, T], bf16)
        q2 = persist.tile([128, T], bf16)        # h2, duplicated to rows 64:128
        k2 = persist.tile([128, T], bf16)
        v_sb = persist.tile([128, TT, HPC, 68], bf16)  # [V|1] per head
        # normalized O^T: h0 rows 0:64, h1 rows 64:128; h2 separate
        on01 = persist.tile([128, T], bf16)
        on2 = persist.tile([DH, T], bf16)
        wo01_sb = persist.tile([128, D], bf16)
        wo2_sb = persist.tile([DH, D], bf16)

        nc.vector.memset(v_sb[:, :, :, 64:65], 1.0)

        # ---- input DMAs ----
        xT, wqT, wkT, wvT, bq, bk, bv, wo01, wo2, onesb, ones65d, y = tens
        nc.sync.dma_start(ones1[:], onesb[0:1, 0:128])
        nc.sync.dma_start(ones65[DH : DH + 1, :], ones65d[:])
        for kc in range(KC):
            r = slice(kc * 128, (kc + 1) * 128)
            nc.sync.dma_start(x_ch[kc][:], xT[r, :])
            nc.sync.dma_start(w_q[:, kc, :], wqT[r, :])
            nc.sync.dma_start(w_k[:, kc, :], wkT[r, :])
            nc.sync.dma_start(w_v[:, kc, :], wvT[r, :])
        nc.sync.dma_start(bq01[:], bq[0:128, :])
        nc.sync.dma_start(bq2[:], bq[128:192, :])
        nc.sync.dma_start(bk01[:], bk[0:128, :])
        nc.sync.dma_start(bk2[:], bk[128:192, :])
        nc.sync.dma_start(bv_sb[:], bv[:])
        nc.sync.dma_start(wo01_sb[:], wo01[:])
        nc.sync.dma_start(wo2_sb[:], wo2[:])

        loop_cm = tc.For_i(0, iters, 1) if iters > 1 else None
        from contextlib import nullcontext
        with (loop_cm if loop_cm is not None else nullcontext()):
            # ---- Phase 1a: Q/K projections into [d, t] layout ----
            # Two t-tiles in flight per weight chunk: LDWEIGHTS amortizes 2x.
            with tc.tile_pool(name="pj", bufs=1, space=PSUM) as pj:
                for np_ in range(NT // 2):
                    nts = (2 * np_, 2 * np_ + 1)
                    ss = [slice(nt * 512, (nt + 1) * 512) for nt in nts]
                    pqa = [pj.tile([128, 512], f32, tag=f"pqa{j}", name=f"pqa{j}")
                           for j in (0, 1)]
                    pka = [pj.tile([128, 512], f32, tag=f"pka{j}", name=f"pka{j}")
                           for j in (0, 1)]
                    pb = [pj.tile([128, 512], f32, tag=f"pb{j}", name=f"pb{j}")
                          for j in (0, 1)]
                    for kc in range(KC):
                        st, sp = kc == 0, kc == KC - 1
                        for j in (0, 1):
                            nc.tensor.matmul(pqa[j][:], w_q[:, kc, 0:128],
                                             x_ch[kc][:, ss[j]], start=st, stop=sp)
                        for j in (0, 1):
                            nc.tensor.matmul(pka[j][:], w_k[:, kc, 0:128],
                                             x_ch[kc][:, ss[j]], start=st, stop=sp)
                        for j in (0, 1):
                            nc.tensor.matmul(pb[j][0:64, :], w_q[:, kc, 128:192],
                                             x_ch[kc][:, ss[j]], start=st, stop=sp,
                                             tile_position=(0, 0), skip_group_check=True)
                        for j in (0, 1):
                            nc.tensor.matmul(pb[j][64:128, :], w_k[:, kc, 128:192],
                                             x_ch[kc][:, ss[j]], start=st, stop=sp,
                                             tile_position=(0, 64), skip_group_check=True)
                    for j in (0, 1):
                        nc.scalar.activation(q01[:, ss[j]], pqa[j][:], Ident,
                                             bias=bq01[:])
                        nc.vector.tensor_scalar_add(k01[:, ss[j]], pka[j][:], bk01[:])
                        nc.scalar.activation(q2[0:64, ss[j]], pb[j][0:64, :], Ident,
                                             bias=bq2[:])
                        nc.vector.tensor_scalar_add(k2[0:64, ss[j]], pb[j][64:128, :], bk2[:])

            # ---- Phase 1b: V projection into natural [t, d] layout ----
            with tc.tile_pool(name="pv", bufs=4, space=PSUM) as pvp:
                for tt in range(TT):
                    ts_ = slice(tt * 128, (tt + 1) * 128)
                    pvt = pvp.tile([128, HPC * DH], f32, tag="pvt")
                    nc.tensor.matmul(pvt[:], ones1[:], bv_sb[:], start=True, stop=False)
                    for kc in range(KC):
                        nc.tensor.matmul(
                            pvt[:], x_ch[kc][:, ts_], w_v[:, kc, :],
                            start=False, stop=kc == KC - 1,
                        )
                    nc.scalar.copy(
                        v_sb[:, tt, :, 0:64],
                        pvt[:].rearrange("p (h d) -> p h d", h=HPC),
                    )

            # duplicate h2's Q/K to partitions 64..127 for self-paired row tiling
            nc.sync.dma_start(q2[64:128, :], q2[0:64, :])
            nc.sync.dma_start(k2[64:128, :], k2[0:64, :])

            if ablate == "p1":
                return

            # ---- Phase 2: attention + output projection, per q tile ----
            # PSUM: spool d0,d1 ([128,1024] = 2 banks each) = 4 banks;
            # opool o0,o1,o2 = 3; mpool aux = 1 (bc broadcasts + Wo blocks).
            with (
                tc.tile_pool(name="spool", bufs=1, space=PSUM) as spool,
                tc.tile_pool(name="opool", bufs=1, space=PSUM) as opool,
                tc.tile_pool(name="mpool", bufs=1, space=PSUM) as mpool,
                tc.tile_pool(name="ppool", bufs=3) as ppool,
                tc.tile_pool(name="npool", bufs=2) as npool,
                tc.tile_pool(name="ypool", bufs=2) as ypool,
            ):
                deferred = []  # per-qtile tail work interleaved into next qtile
                pend = []      # PV matmuls, flushed one group behind (carried
                               # across q-tile boundaries to avoid a PE drain
                               # stall at each boundary)

                def flush(n):
                    while len(pend) > n:
                        out, lhsT, rhs, st_, sp_ = pend.pop(0)
                        nc.tensor.matmul(out, lhsT, rhs, start=st_,
                                         stop=sp_, skip_group_check=True)

                for qt in range(NT):
                    qs = slice(qt * 512, (qt + 1) * 512)

                    ol = [opool.tile([DH + 1, 512], f32, tag=f"o{i}", name=f"ol{i}")
                          for i in (0, 1)]
                    o2 = opool.tile([DH + 1, 512], f32, tag="o2", name="o2")

                    def norm(o_acc, out_ap):
                        # denom row -> SBUF (ACT), K=1 broadcast (PE, f32r),
                        # reciprocal (DVE), one multiply (DVE).
                        lrow = npool.tile([DH + 1, 512], f32r, tag="lr")
                        nc.scalar.copy(lrow[DH : DH + 1, :], o_acc[DH : DH + 1, :])
                        bct = mpool.tile([128, 512], f32, tag="aux", name="bct")
                        bc = bct[0 : DH + 1, :]
                        nc.tensor.matmul(bc, ones65[DH : DH + 1, :],
                                         lrow[DH : DH + 1, :], start=True, stop=True)
                        rc = npool.tile([DH + 1, 512], f32, tag="rc")
                        nc.vector.reciprocal_approx_fast(rc[:], bc)
                        nc.vector.tensor_mul(out_ap, o_acc[0:DH, :], rc[0:DH, :])

                    # 24 groups of 2 steps; PV batched one group behind.
                    for g in range(24):
                        for j in (0, 1):
                            step = 2 * g + j
                            dt_ = spool.tile([128, 1024], f32, tag=f"d{j}",
                                             name=f"d{j}")
                            if step < TT:
                                kt = step
                                ks = slice(kt * 128, (kt + 1) * 128)
                                nc.tensor.matmul(dt_[:, 0:512], k01[0:64, ks],
                                                 q01[0:64, qs], start=True, stop=True,
                                                 skip_group_check=True)
                                nc.tensor.matmul(dt_[:, 512:1024], k01[64:128, ks],
                                                 q01[64:128, qs], start=True, stop=True,
                                                 skip_group_check=True)
                            else:
                                p = step - TT
                                ka = slice(2 * p * 128, (2 * p + 1) * 128)
                                kb = slice((2 * p + 1) * 128, (2 * p + 2) * 128)
                                nc.tensor.matmul(dt_[:, 0:512], k2[0:64, ka],
                                                 q2[0:64, qs], start=True, stop=True,
                                                 skip_group_check=True)
                                nc.tensor.matmul(dt_[:, 512:1024], k2[64:128, kb],
                                                 q2[64:128, qs], start=True, stop=True,
                                                 skip_group_check=True)
                            pt = ppool.tile([128, 1024], bf16, tag=f"p{j}",
                                            name=f"p{j}")
                            if step % 2 == 0:
                                nc.scalar.activation(pt[:, 0:XSPL], dt_[:, 0:XSPL],
                                                     Exp, scale=0.125)
                                nc.vector.tensor_scalar(
                                    pt[:, XSPL:1024].bitcast(i16), dt_[:, XSPL:1024],
                                    SCH_A, SCH_B, Mult, Add)
                            else:
                                nc.vector.tensor_scalar(
                                    pt[:, 0:1024 - XSPL].bitcast(i16),
                                    dt_[:, 0:1024 - XSPL],
                                    SCH_A, SCH_B, Mult, Add)
                                nc.scalar.activation(pt[:, 1024 - XSPL:1024],
                                                     dt_[:, 1024 - XSPL:1024],
                                                     Exp, scale=0.125)
                            if step < TT:
                                kt = step
                                fi, la = kt == 0, kt == TT - 1
                                pend.append((ol[0][:], v_sb[:, kt, 0, 0:65],
                                             pt[:, 0:512], fi, la))
                                pend.append((ol[1][:], v_sb[:, kt, 1, 0:65],
                                             pt[:, 512:1024], fi, la))
                            else:
                                p = step - TT
                                pend.append((o2[:], v_sb[:, 2 * p, 2, 0:65],
                                             pt[:, 0:512], p == 0, False))
                                pend.append((o2[:], v_sb[:, 2 * p + 1, 2, 0:65],
                                             pt[:, 512:1024], False, p == TT // 2 - 1))
                        flush(6)
                        if deferred and 1 <= g <= 12:
                            deferred.pop(0)()
                        if g == 17:
                            # h0/h1 PV chains drained during group 16's flush;
                            # normalize them while the h2 groups continue.
                            norm(ol[0], on01[0:DH, qs])
                        if g == 19:
                            norm(ol[1], on01[DH:128, qs])

                    def defer_norm_o2(o2=o2, qs=qs):
                        norm(o2, on2[0:DH, qs])
                    deferred.append(defer_norm_o2)

                    def defer_wo(tt4, qt=qt):
                        t0 = qt * 512 + tt4 * 128
                        ts_ = slice(t0, t0 + 128)
                        ysb = ypool.tile([128, D], bf16, tag="ysb", name="ysb")
                        for m0, mw in ((0, 512), (512, 256)):
                            ms = slice(m0, m0 + mw)
                            yps = mpool.tile([128, 512], f32, tag="aux", name="yps")
                            nc.tensor.matmul(yps[:, 0:mw], on01[:, ts_], wo01_sb[:, ms],
                                             start=True, stop=False)
                            nc.tensor.matmul(yps[:, 0:mw], on2[:, ts_], wo2_sb[:, ms],
                                             start=False, stop=True)
                            nc.scalar.copy(ysb[:, ms], yps[:, 0:mw])
                        nc.sync.dma_start(y[ts_, :], ysb[:])

                    for tt4 in range(4):
                        deferred.append(lambda tt4=tt4: defer_wo(tt4))

                # drain the last qtile's PVs and tail
                flush(0)
                for f in deferred:
                    f()


def _build(iters=1):
    import concourse.bacc as bacc
    import concourse.tile as tile
    from concourse import mybir

    f32 = mybir.dt.float32
    bf16 = mybir.dt.bfloat16
    nc = bacc.Bacc("TRN2", target_bir_lowering=False, debug=False, name="mhsa")

    tens = (
        nc.dram_tensor("xT", [D, T], bf16, kind="ExternalInput"),
        nc.dram_tensor("wqT", [D, HPC * DH], bf16, kind="ExternalInput"),
        nc.dram_tensor("wkT", [D, HPC * DH], bf16, kind="ExternalInput"),
        nc.dram_tensor("wvT", [D, HPC * DH], bf16, kind="ExternalInput"),
        nc.dram_tensor("bq", [HPC * DH, 1], f32, kind="ExternalInput"),
        nc.dram_tensor("bk", [HPC * DH, 1], f32, kind="ExternalInput"),
        nc.dram_tensor("bv", [1, HPC * DH], bf16, kind="ExternalInput"),
        nc.dram_tensor("wo01", [128, D], bf16, kind="ExternalInput"),
        nc.dram_tensor("wo2", [DH, D], bf16, kind="ExternalInput"),
        nc.dram_tensor("onesb", [1, T], bf16, kind="ExternalInput"),
        nc.dram_tensor("ones65", [1, DH + 1], mybir.dt.float32r, kind="ExternalInput"),
        nc.dram_tensor("y", [T, D], bf16, kind="ExternalOutput"),
    )
    with tile.TileContext(nc) as tc:
        _trace(nc, tc, mybir, tens, iters)
    nc.finalize()
    return nc


def _prep_inputs(x, Wq, bq, Wk, bk, Wv, bv, Wo, bo):
    in_maps = []
    xTb = [np.ascontiguousarray(x[b].T).astype(BF16) for b in range(B)]
    for c in range(NCORES):
        b = c // 4
        h0 = (c % 4) * HPC
        cols = slice(h0 * DH, (h0 + HPC) * DH)
        woT = np.ascontiguousarray(Wo[:, cols].T)  # [192, 768]
        wo01 = np.ascontiguousarray(woT[0:128]).astype(BF16)
        wo2 = np.ascontiguousarray(woT[128:192]).astype(BF16)
        in_maps.append(
            {
                "xT": xTb[b],
                "wqT": np.ascontiguousarray(Wq[cols, :].T).astype(BF16),
                "wkT": np.ascontiguousarray(Wk[cols, :].T).astype(BF16),
                "wvT": np.ascontiguousarray(Wv[cols, :].T).astype(BF16),
                "bq": np.ascontiguousarray(bq[cols]).reshape(-1, 1).astype(np.float32),
                "bk": np.ascontiguousarray(bk[cols]).reshape(-1, 1).astype(np.float32),
                "bv": np.ascontiguousarray(bv[cols]).reshape(1, -1).astype(BF16),
                "wo01": wo01,
                "wo2": wo2,
                "onesb": np.ones((1, T), dtype=BF16),
                "ones65": np.ones((1, DH + 1), dtype=np.float32),
            }
        )
    return in_maps


def kernel(x, Wq, bq, Wk, bk, Wv, bv, Wo, bo):
    x = np.asarray(x, dtype=np.float32)
    Wq, bq = np.asarray(Wq, np.float32), np.asarray(bq, np.float32)
    Wk, bk = np.asarray(Wk, np.float32), np.asarray(bk, np.float32)
    Wv, bv = np.asarray(Wv, np.float32), np.asarray(bv, np.float32)
    Wo, bo = np.asarray(Wo, np.float32), np.asarray(bo, np.float32)

    from concourse.bass_utils import run_bass_kernel_spmd

    iters = int(os.environ.get("MHSA_ITERS", "1"))
    key = ("nc", iters, os.environ.get("MHSA_ABLATE", ""),
           os.environ.get("MHSA_X", ""))
    if key not in _CACHE:
        _CACHE[key] = _build(iters)
    nc = _CACHE[key]

    in_maps = _prep_inputs(x, Wq, bq, Wk, bk, Wv, bv, Wo, bo)
    trace = bool(os.environ.get("MHSA_TRACE"))
    ncores = int(os.environ.get("MHSA_NCORES", NCORES))
    res = run_bass_kernel_spmd(
        nc, in_maps[:ncores], core_ids=list(range(ncores)), trace=trace
    )
    if res.exec_time_ns is not None:
        print(f"HW exec time: {res.exec_time_ns} ns")
        _CACHE["exec_time_ns"] = res.exec_time_ns
        _CACHE["trace"] = res.instructions_and_trace

    out = np.zeros((B, T, D), dtype=np.float32)
    for c in range(ncores):
        out[c // 4] += res.results[c]["y"].astype(np.float32)
    out += bo[None, None, :]
    return out


# revision 22
# speedup vs baseline: 1.0528x; 1.0528x over previous
"""Multi-head self-attention (B=2, T=4096, D=768, H=12) on 8 TRN2 NeuronCores.

Sharding: (batch, head)-parallel. Core c (0..7) handles batch b=c//4 and the
3 heads h0=(c%4)*3 .. h0+2.  Each core computes Q/K/V projections for its
heads, full softmax(QK^T/sqrt(d))V attention, and a partial output projection
through its 192 rows of Wo.  The host sums the 4 partials per batch and adds
the output bias bo.

Per-core pipeline (v5):
  Phase 1: Q/K projections into [d, t] layout; two t-tiles interleaved per
  weight chunk so LDWEIGHTS amortizes 2x; bias adds on ACT.  V in natural
  [t, d] layout with a trailing ones column (row 64 of the PV output
  accumulates the softmax denominator); V copies on ACT.
  Phase 2, per 512-wide q tile: 24 groups of 2 k-steps.  Each step: a
  row-tiled pair of score MMs (S^T[k,q] for 2 head-slots) into ONE 2-bank
  [128,1024] PSUM tile; exp split asymmetrically between ACT (exact exp,
  front XSPL cols on even steps / back XSPL on odd) and DVE (Schraudolph
  bitcast exp: i16 = round(23.083*s + 16250.5) = bf16 e^(s/8), ~3%
  pointwise) -- the DVE's effective throughput is ~half its op cost (pipe
  drain), so it gets the smaller share.  The previous group's 4 PV MMs run
  batched (fewer LDWEIGHTS transitions).  h0/h1 use groups 0-15; h2 uses
  groups 16-23 (Q/K duplicated onto both row halves, 2 k-tiles/step).
  Normalization: denominator row -> SBUF (ACT), fast reciprocal on [1,512]
  (DVE), K=1 broadcast matmul of the reciprocal, then one multiply (DVE).
  Wo accumulates in PSUM and is DMAed to HBM as f32 directly (no copy).
"""

import os
import numpy as np
import ml_dtypes

B, T, D = 2, 4096, 768
H, DH = 12, 64
NCORES = 8
HPC = 3            # heads per core
KC = D // 128      # 6 contraction chunks for projections
NT = T // 512      # 8 q tiles of 512
TT = T // 128      # 32 k tiles of 128

# Schraudolph constants: i16 = round(s * SCH_A + SCH_B); bits viewed as bf16
# give e^(s/8) with ~3.3% max pointwise error.
SCH_A = 128 / float(np.log(2)) * 0.125   # 23.08312065
SCH_B = 16256.0 - 5.5

BF16 = ml_dtypes.bfloat16

_CACHE = {}


def _trace(nc, tc, mybir, tens, iters=1):
    import concourse.bass as bass
    from contextlib import ExitStack

    ablate = os.environ.get("MHSA_ABLATE", "")
    XSPL = 512 + int(os.environ.get("MHSA_X", "192"))  # ACT's exp share

    f32 = mybir.dt.float32
    bf16 = mybir.dt.bfloat16
    i16 = mybir.dt.int16
    f32r = mybir.dt.float32r
    Exp = mybir.ActivationFunctionType.Exp
    Ident = mybir.ActivationFunctionType.Identity
    PSUM = bass.MemorySpace.PSUM
    Mult = mybir.AluOpType.mult
    Add = mybir.AluOpType.add

    with ExitStack() as ctx:
        persist = ctx.enter_context(tc.tile_pool(name="persist", bufs=1))

        # ---- persistent SBUF ----
        x_ch = [
            persist.tile([128, T], bf16, name=f"xc{kc}") for kc in range(KC)
        ]
        w_q = persist.tile([128, KC, HPC * DH], bf16)
        w_k = persist.tile([128, KC, HPC * DH], bf16)
        w_v = persist.tile([128, KC, HPC * DH], bf16)
        bq01 = persist.tile([128, 1], f32)
        bq2 = persist.tile([64, 1], f32)
        bk01 = persist.tile([128, 1], f32)
        bk2 = persist.tile([64, 1], f32)
        bv_sb = persist.tile([1, HPC * DH], bf16)
        ones1 = persist.tile([1, 128], bf16)     # K=1 lhsT for V bias MM
        ones65 = persist.tile([DH + 1, DH + 1], f32r)  # row 64: K=1 denom bcast lhsT
        q01 = persist.tile([128, T], bf16)       # h0 rows 0:64, h1 rows 64:128
        k01 = persist.tile([128, T], bf16)
        q2 = persist.tile([128, T], bf16)        # h2, duplicated to rows 64:128
        k2 = persist.tile([128, T], bf16)
        v_sb = persist.tile([128, TT, HPC, 68], bf16)  # [V|1] per head
        # normalized O^T: h0 rows 0:64, h1 rows 64:128; h2 separate
        on01 = persist.tile([128, T], bf16)
        on2 = persist.tile([DH, T], bf16)
        wo01_sb = persist.tile([128, D], bf16)
        wo2_sb = persist.tile([DH, D], bf16)

        nc.vector.memset(v_sb[:, :, :, 64:65], 1.0)

        # ---- input DMAs ----
        xT, wqT, wkT, wvT, bq, bk, bv, wo01, wo2, onesb, ones65d, y = tens
        nc.sync.dma_start(ones1[:], onesb[0:1, 0:128])
        nc.sync.dma_start(ones65[DH : DH + 1, :], ones65d[:])
        for kc in range(KC):
            r = slice(kc * 128, (kc + 1) * 128)
            nc.sync.dma_start(x_ch[kc][:], xT[r, :])
            nc.sync.dma_start(w_q[:, kc, :], wqT[r, :])
            nc.sync.dma_start(w_k[:, kc, :], wkT[r, :])
            nc.sync.dma_start(w_v[:, kc, :], wvT[r, :])
        nc.sync.dma_start(bq01[:], bq[0:128, :])
        nc.sync.dma_start(bq2[:], bq[128:192, :])
        nc.sync.dma_start(bk01[:], bk[0:128, :])
        nc.sync.dma_start(bk2[:], bk[128:192, :])
        nc.sync.dma_start(bv_sb[:], bv[:])
        nc.sync.dma_start(wo01_sb[:], wo01[:])
        nc.sync.dma_start(wo2_sb[:], wo2[:])

        loop_cm = tc.For_i(0, iters, 1) if iters > 1 else None
        from contextlib import nullcontext
        with (loop_cm if loop_cm is not None else nullcontext()):
            # ---- Phase 1a: Q/K projections into [d, t] layout ----
            # Two t-tiles in flight per weight chunk: LDWEIGHTS amortizes 2x.
            with tc.tile_pool(name="pj", bufs=1, space=PSUM) as pj:
                for np_ in range(NT // 2):
                    nts = (2 * np_, 2 * np_ + 1)
                    ss = [slice(nt * 512, (nt + 1) * 512) for nt in nts]
                    pqa = [pj.tile([128, 512], f32, tag=f"pqa{j}", name=f"pqa{j}")
                           for j in (0, 1)]
                    pka = [pj.tile([128, 512], f32, tag=f"pka{j}", name=f"pka{j}")
                           for j in (0, 1)]
                    pb = [pj.tile([128, 512], f32, tag=f"pb{j}", name=f"pb{j}")
                          for j in (0, 1)]
                    for kc in range(KC):
                        st, sp = kc == 0, kc == KC - 1
                        for j in (0, 1):
                            nc.tensor.matmul(pqa[j][:], w_q[:, kc, 0:128],
                                             x_ch[kc][:, ss[j]], start=st, stop=sp)
                        for j in (0, 1):
                            nc.tensor.matmul(pka[j][:], w_k[:, kc, 0:128],
                                             x_ch[kc][:, ss[j]], start=st, stop=sp)
                        for j in (0, 1):
                            nc.tensor.matmul(pb[j][0:64, :], w_q[:, kc, 128:192],
                                             x_ch[kc][:, ss[j]], start=st, stop=sp,
                                             tile_position=(0, 0), skip_group_check=True)
                        for j in (0, 1):
                            nc.tensor.matmul(pb[j][64:128, :], w_k[:, kc, 128:192],
                                             x_ch[kc][:, ss[j]], start=st, stop=sp,
                                             tile_position=(0, 64), skip_group_check=True)
                    for j in (0, 1):
                        nc.scalar.activation(q01[:, ss[j]], pqa[j][:], Ident,
                                             bias=bq01[:])
                        nc.scalar.activation(k01[:, ss[j]], pka[j][:], Ident,
                                             bias=bk01[:])
                        nc.scalar.activation(q2[0:64, ss[j]], pb[j][0:64, :], Ident,
                                             bias=bq2[:])
                        nc.vector.tensor_scalar_add(k2[0:64, ss[j]], pb[j][64:128, :], bk2[:])

            # ---- Phase 1b: V projection into natural [t, d] layout ----
            with tc.tile_pool(name="pv", bufs=4, space=PSUM) as pvp:
                for tt in range(TT):
                    ts_ = slice(tt * 128, (tt + 1) * 128)
                    pvt = pvp.tile([128, HPC * DH], f32, tag="pvt")
                    nc.tensor.matmul(pvt[:], ones1[:], bv_sb[:], start=True, stop=False)
                    for kc in range(KC):
                        nc.tensor.matmul(
                            pvt[:], x_ch[kc][:, ts_], w_v[:, kc, :],
                            start=False, stop=kc == KC - 1,
                        )
                    nc.scalar.copy(
                        v_sb[:, tt, :, 0:64],
                        pvt[:].rearrange("p (h d) -> p h d", h=HPC),
                    )

            # duplicate h2's Q/K to partitions 64..127 for self-paired row tiling
            nc.sync.dma_start(q2[64:128, :], q2[0:64, :])
            nc.sync.dma_start(k2[64:128, :], k2[0:64, :])

            if ablate == "p1":
                return

            # ---- Phase 2: attention + output projection, per q tile ----
            # PSUM: spool d0,d1 ([128,1024] = 2 banks each) = 4 banks;
            # opool o0,o1,o2 = 3; mpool aux = 1 (bc broadcasts + Wo blocks).
            with (
                tc.tile_pool(name="spool", bufs=1, space=PSUM) as spool,
                tc.tile_pool(name="opool", bufs=1, space=PSUM) as opool,
                tc.tile_pool(name="mpool", bufs=1, space=PSUM) as mpool,
                tc.tile_pool(name="ppool", bufs=3) as ppool,
                tc.tile_pool(name="npool", bufs=2) as npool,
                tc.tile_pool(name="ypool", bufs=2) as ypool,
            ):
                deferred = []  # per-qtile tail work interleaved into next qtile
                pend = []      # PV matmuls, flushed one group behind (carried
                               # across q-tile boundaries to avoid a PE drain
                               # stall at each boundary)

                def flush(n):
                    while len(pend) > n:
                        out, lhsT, rhs, st_, sp_ = pend.pop(0)
                        nc.tensor.matmul(out, lhsT, rhs, start=st_,
                                         stop=sp_, skip_group_check=True)

                for qt in range(NT):
                    qs = slice(qt * 512, (qt + 1) * 512)

                    ol = [opool.tile([DH + 1, 512], f32, tag=f"o{i}", name=f"ol{i}")
                          for i in (0, 1)]
                    o2 = opool.tile([DH + 1, 512], f32, tag="o2", name="o2")

                    def norm(o_acc, out_ap):
                        # denom row -> SBUF (ACT), K=1 broadcast (PE, f32r),
                        # reciprocal (DVE), one multiply (DVE).
                        lrow = npool.tile([DH + 1, 512], f32r, tag="lr")
                        nc.scalar.copy(lrow[DH : DH + 1, :], o_acc[DH : DH + 1, :])
                        bct = mpool.tile([128, 512], f32, tag="aux", name="bct")
                        bc = bct[0 : DH + 1, :]
                        nc.tensor.matmul(bc, ones65[DH : DH + 1, :],
                                         lrow[DH : DH + 1, :], start=True, stop=True)
                        rc = npool.tile([DH + 1, 512], f32, tag="rc")
                        nc.vector.reciprocal_approx_fast(rc[:], bc)
                        nc.vector.tensor_mul(out_ap, o_acc[0:DH, :], rc[0:DH, :])

                    # 24 groups of 2 steps; PV batched one group behind.
                    for g in range(24):
                        for j in (0, 1):
                            step = 2 * g + j
                            dt_ = spool.tile([128, 1024], f32, tag=f"d{j}",
                                             name=f"d{j}")
                            if step < TT:
                                kt = step
                                ks = slice(kt * 128, (kt + 1) * 128)
                                nc.tensor.matmul(dt_[:, 0:512], k01[0:64, ks],
                                                 q01[0:64, qs], start=True, stop=True,
                                                 skip_group_check=True)
                                nc.tensor.matmul(dt_[:, 512:1024], k01[64:128, ks],
                                                 q01[64:128, qs], start=True, stop=True,
                                                 skip_group_check=True)
                            else:
                                p = step - TT
                                ka = slice(2 * p * 128, (2 * p + 1) * 128)
                                kb = slice((2 * p + 1) * 128, (2 * p + 2) * 128)
                                nc.tensor.matmul(dt_[:, 0:512], k2[0:64, ka],
                                                 q2[0:64, qs], start=True, stop=True,
                                                 skip_group_check=True)
                                nc.tensor.matmul(dt_[:, 512:1024], k2[64:128, kb],
                                                 q2[64:128, qs], start=True, stop=True,
                                                 skip_group_check=True)
                            pt = ppool.tile([128, 1024], bf16, tag=f"p{j}",
                                            name=f"p{j}")
                            if step % 2 == 0:
                                nc.scalar.activation(pt[:, 0:XSPL], dt_[:, 0:XSPL],
                                                     Exp, scale=0.125)
                                nc.vector.tensor_scalar(
                                    pt[:, XSPL:1024].bitcast(i16), dt_[:, XSPL:1024],
                                    SCH_A, SCH_B, Mult, Add)
                            else:
                                nc.vector.tensor_scalar(
                                    pt[:, 0:1024 - XSPL].bitcast(i16),
                                    dt_[:, 0:1024 - XSPL],
                                    SCH_A, SCH_B, Mult, Add)
                                nc.scalar.activation(pt[:, 1024 - XSPL:1024],
                                                     dt_[:, 1024 - XSPL:1024],
                                                     Exp, scale=0.125)
                            if step < TT:
                                kt = step
                                fi, la = kt == 0, kt == TT - 1
                                pend.append((ol[0][:], v_sb[:, kt, 0, 0:65],
                                             pt[:, 0:512], fi, la))
                                pend.append((ol[1][:], v_sb[:, kt, 1, 0:65],
                                             pt[:, 512:1024], fi, la))
                            else:
                                p = step - TT
                                pend.append((o2[:], v_sb[:, 2 * p, 2, 0:65],
                                             pt[:, 0:512], p == 0, False))
                                pend.append((o2[:], v_sb[:, 2 * p + 1, 2, 0:65],
                                             pt[:, 512:1024], False, p == TT // 2 - 1))
                        flush(6)
                        if deferred and 1 <= g <= 12 and g % 2 == 1:
                            deferred.pop(0)()
                        if g == 17:
                            # h0/h1 PV chains drained during group 16's flush;
                            # normalize them while the h2 groups continue.
                            norm(ol[0], on01[0:DH, qs])
                        if g == 19:
                            norm(ol[1], on01[DH:128, qs])

                    def defer_norm_o2(o2=o2, qs=qs):
                        norm(o2, on2[0:DH, qs])
                    deferred.append(defer_norm_o2)

                    def defer_wo(tt4, qt=qt):
                        t0 = qt * 512 + tt4 * 128
                        ts_ = slice(t0, t0 + 128)
                        ysb = ypool.tile([128, D], bf16, tag="ysb", name="ysb")
                        for m0, mw in ((0, 512), (512, 256)):
                            ms = slice(m0, m0 + mw)
                            yps = mpool.tile([128, 512], f32, tag="aux", name="yps")
                            nc.tensor.matmul(yps[:, 0:mw], on01[:, ts_], wo01_sb[:, ms],
                                             start=True, stop=False)
                            nc.tensor.matmul(yps[:, 0:mw], on2[:, ts_], wo2_sb[:, ms],
                                             start=False, stop=True)
                            nc.scalar.copy(ysb[:, ms], yps[:, 0:mw])
                        nc.sync.dma_start(y[ts_, :], ysb[:])

                    for tt4 in range(4):
                        deferred.append(lambda tt4=tt4: defer_wo(tt4))

                # drain the last qtile's PVs and tail
                flush(0)
                for f in deferred:
                    f()


def _build(iters=1):
    import concourse.bacc as bacc
    import concourse.tile as tile
    from concourse import mybir

    f32 = mybir.dt.float32
    bf16 = mybir.dt.bfloat16
    nc = bacc.Bacc("TRN2", target_bir_lowering=False, debug=False, name="mhsa")

    tens = (
        nc.dram_tensor("xT", [D, T], bf16, kind="ExternalInput"),
        nc.dram_tensor("wqT", [D, HPC * DH], bf16, kind="ExternalInput"),
        nc.dram_tensor("wkT", [D, HPC * DH], bf16, kind="ExternalInput"),
        nc.dram_tensor("wvT", [D, HPC * DH], bf16, kind="ExternalInput"),
        nc.dram_tensor("bq", [HPC * DH, 1], f32, kind="ExternalInput"),
        nc.dram_tensor("bk", [HPC * DH, 1], f32, kind="ExternalInput"),
        nc.dram_tensor("bv", [1, HPC * DH], bf16, kind="ExternalInput"),
        nc.dram_tensor("wo01", [128, D], bf16, kind="ExternalInput"),
        nc.dram_tensor("wo2", [DH, D], bf16, kind="ExternalInput"),
        nc.dram_tensor("onesb", [1, T], bf16, kind="ExternalInput"),
        nc.dram_tensor("ones65", [1, DH + 1], mybir.dt.float32r, kind="ExternalInput"),
        nc.dram_tensor("y", [T, D], bf16, kind="ExternalOutput"),
    )
    with tile.TileContext(nc) as tc:
        _trace(nc, tc, mybir, tens, iters)
    nc.finalize()
    return nc


def _prep_inputs(x, Wq, bq, Wk, bk, Wv, bv, Wo, bo):
    in_maps = []
    xTb = [np.ascontiguousarray(x[b].T).astype(BF16) for b in range(B)]
    for c in range(NCORES):
        b = c // 4
        h0 = (c % 4) * HPC
        cols = slice(h0 * DH, (h0 + HPC) * DH)
        woT = np.ascontiguousarray(Wo[:, cols].T)  # [192, 768]
        wo01 = np.ascontiguousarray(woT[0:128]).astype(BF16)
        wo2 = np.ascontiguousarray(woT[128:192]).astype(BF16)
        in_maps.append(
            {
                "xT": xTb[b],
                "wqT": np.ascontiguousarray(Wq[cols, :].T).astype(BF16),
                "wkT": np.ascontiguousarray(Wk[cols, :].T).astype(BF16),
                "wvT": np.ascontiguousarray(Wv[cols, :].T).astype(BF16),
                "bq": np.ascontiguousarray(bq[cols]).reshape(-1, 1).astype(np.float32),
                "bk": np.ascontiguousarray(bk[cols]).reshape(-1, 1).astype(np.float32),
                "bv": np.ascontiguousarray(bv[cols]).reshape(1, -1).astype(BF16),
                "wo01": wo01,
                "wo2": wo2,
                "onesb": np.ones((1, T), dtype=BF16),
                "ones65": np.ones((1, DH + 1), dtype=np.float32),
            }
        )
    return in_maps


def kernel(x, Wq, bq, Wk, bk, Wv, bv, Wo, bo):
    x = np.asarray(x, dtype=np.float32)
    Wq, bq = np.asarray(Wq, np.float32), np.asarray(bq, np.float32)
    Wk, bk = np.asarray(Wk, np.float32), np.asarray(bk, np.float32)
    Wv, bv = np.asarray(Wv, np.float32), np.asarray(bv, np.float32)
    Wo, bo = np.asarray(Wo, np.float32), np.asarray(bo, np.float32)

    from concourse.bass_utils import run_bass_kernel_spmd

    iters = int(os.environ.get("MHSA_ITERS", "1"))
    key = ("nc", iters, os.environ.get("MHSA_ABLATE", ""),
           os.environ.get("MHSA_X", ""))
    if key not in _CACHE:
        _CACHE[key] = _build(iters)
    nc = _CACHE[key]

    in_maps = _prep_inputs(x, Wq, bq, Wk, bk, Wv, bv, Wo, bo)
    trace = bool(os.environ.get("MHSA_TRACE"))
    ncores = int(os.environ.get("MHSA_NCORES", NCORES))
    res = run_bass_kernel_spmd(
        nc, in_maps[:ncores], core_ids=list(range(ncores)), trace=trace
    )
    if res.exec_time_ns is not None:
        print(f"HW exec time: {res.exec_time_ns} ns")
        _CACHE["exec_time_ns"] = res.exec_time_ns
        _CACHE["trace"] = res.instructions_and_trace

    out = np.zeros((B, T, D), dtype=np.float32)
    for c in range(ncores):
        out[c // 4] += res.results[c]["y"].astype(np.float32)
    out += bo[None, None, :]
    return out


# revision 23
# speedup vs baseline: 1.0807x; 1.0265x over previous
"""Multi-head self-attention (B=2, T=4096, D=768, H=12) on 8 TRN2 NeuronCores.

Sharding: (batch, head)-parallel. Core c (0..7) handles batch b=c//4 and the
3 heads h0=(c%4)*3 .. h0+2.  Each core computes Q/K/V projections for its
heads, full softmax(QK^T/sqrt(d))V attention, and a partial output projection
through its 192 rows of Wo.  The host sums the 4 partials per batch and adds
the output bias bo.

Per-core pipeline (v5):
  Phase 1: Q/K projections into [d, t] layout; two t-tiles interleaved per
  weight chunk so LDWEIGHTS amortizes 2x; bias adds on ACT.  V in natural
  [t, d] layout with a trailing ones column (row 64 of the PV output
  accumulates the softmax denominator); V copies on ACT.
  Phase 2, per 512-wide q tile: 24 groups of 2 k-steps.  Each step: a
  row-tiled pair of score MMs (S^T[k,q] for 2 head-slots) into ONE 2-bank
  [128,1024] PSUM tile; exp split asymmetrically between ACT (exact exp,
  front XSPL cols on even steps / back XSPL on odd) and DVE (Schraudolph
  bitcast exp: i16 = round(23.083*s + 16250.5) = bf16 e^(s/8), ~3%
  pointwise) -- the DVE's effective throughput is ~half its op cost (pipe
  drain), so it gets the smaller share.  The previous group's 4 PV MMs run
  batched (fewer LDWEIGHTS transitions).  h0/h1 use groups 0-15; h2 uses
  groups 16-23 (Q/K duplicated onto both row halves, 2 k-tiles/step).
  The PV queue is carried across q-tile boundaries, and each q-tile's tail
  (norm(o2) + 4 Wo blocks) is deferred into the next q-tile's groups so the
  in-order PE never drains on it.  Normalization: denominator row -> SBUF
  (ACT), K=1 f32r broadcast matmul (PE), fast reciprocal (DVE), one
  multiply (DVE).  Wo accumulates in PSUM, is cast to bf16 on ACT, and
  DMAed to HBM.
"""

import os
import numpy as np
import ml_dtypes

B, T, D = 2, 4096, 768
H, DH = 12, 64
NCORES = 8
HPC = 3            # heads per core
KC = D // 128      # 6 contraction chunks for projections
NT = T // 512      # 8 q tiles of 512
TT = T // 128      # 32 k tiles of 128

# Schraudolph constants: i16 = round(s * SCH_A + SCH_B); bits viewed as bf16
# give e^(s/8) with ~3.3% max pointwise error.
SCH_A = 128 / float(np.log(2)) * 0.125   # 23.08312065
SCH_B = 16256.0 - 5.5

BF16 = ml_dtypes.bfloat16

_CACHE = {}


def _trace(nc, tc, mybir, tens, iters=1):
    import concourse.bass as bass
    from contextlib import ExitStack

    ablate = os.environ.get("MHSA_ABLATE", "")
    XSPL = 512 + int(os.environ.get("MHSA_X", "192"))  # ACT's exp share

    f32 = mybir.dt.float32
    bf16 = mybir.dt.bfloat16
    i16 = mybir.dt.int16
    f32r = mybir.dt.float32r
    Exp = mybir.ActivationFunctionType.Exp
    Ident = mybir.ActivationFunctionType.Identity
    PSUM = bass.MemorySpace.PSUM
    Mult = mybir.AluOpType.mult
    Add = mybir.AluOpType.add

    with ExitStack() as ctx:
        persist = ctx.enter_context(tc.tile_pool(name="persist", bufs=1))

        # ---- persistent SBUF ----
        x_ch = [
            persist.tile([128, T], bf16, name=f"xc{kc}") for kc in range(KC)
        ]
        w_q = persist.tile([128, KC, HPC * DH], bf16)
        w_k = persist.tile([128, KC, HPC * DH], bf16)
        w_v = persist.tile([128, KC, HPC * DH], bf16)
        bq01 = persist.tile([128, 1], f32)
        bq2 = persist.tile([64, 1], f32)
        bk01 = persist.tile([128, 1], f32)
        bk2 = persist.tile([64, 1], f32)
        bv_sb = persist.tile([1, HPC * DH], bf16)
        ones1 = persist.tile([1, 128], bf16)     # K=1 lhsT for V bias MM
        ones65 = persist.tile([DH + 1, DH + 1], f32r)  # row 64: K=1 denom bcast lhsT
        q01 = persist.tile([128, T], bf16)       # h0 rows 0:64, h1 rows 64:128
        k01 = persist.tile([128, T], bf16)
        q2 = persist.tile([128, T], bf16)        # h2, duplicated to rows 64:128
        k2 = persist.tile([128, T], bf16)
        v_sb = persist.tile([128, TT, HPC, 68], bf16)  # [V|1] per head
        # normalized O^T: h0 rows 0:64, h1 rows 64:128; h2 separate
        on01 = persist.tile([128, T], bf16)
        on2 = persist.tile([DH, T], bf16)
        wo01_sb = persist.tile([128, D], bf16)
        wo2_sb = persist.tile([DH, D], bf16)

        nc.vector.memset(v_sb[:, :, :, 64:65], 1.0)

        # ---- input DMAs ----
        xT, wqT, wkT, wvT, bq, bk, bv, wo01, wo2, onesb, ones65d, y = tens
        nc.sync.dma_start(ones1[:], onesb[0:1, 0:128])
        nc.sync.dma_start(ones65[DH : DH + 1, :], ones65d[:])
        for kc in range(KC):
            r = slice(kc * 128, (kc + 1) * 128)
            nc.sync.dma_start(x_ch[kc][:], xT[r, :])
            nc.sync.dma_start(w_q[:, kc, :], wqT[r, :])
            nc.sync.dma_start(w_k[:, kc, :], wkT[r, :])
            nc.sync.dma_start(w_v[:, kc, :], wvT[r, :])
        nc.sync.dma_start(bq01[:], bq[0:128, :])
        nc.sync.dma_start(bq2[:], bq[128:192, :])
        nc.sync.dma_start(bk01[:], bk[0:128, :])
        nc.sync.dma_start(bk2[:], bk[128:192, :])
        nc.sync.dma_start(bv_sb[:], bv[:])
        nc.sync.dma_start(wo01_sb[:], wo01[:])
        nc.sync.dma_start(wo2_sb[:], wo2[:])

        loop_cm = tc.For_i(0, iters, 1) if iters > 1 else None
        from contextlib import nullcontext
        with (loop_cm if loop_cm is not None else nullcontext()):
            # ---- Phase 1a: Q/K projections into [d, t] layout ----
            # Two t-tiles in flight per weight chunk: LDWEIGHTS amortizes 2x.
            with tc.tile_pool(name="pj", bufs=1, space=PSUM) as pj:
                for np_ in range(NT // 2):
                    nts = (2 * np_, 2 * np_ + 1)
                    ss = [slice(nt * 512, (nt + 1) * 512) for nt in nts]
                    pqa = [pj.tile([128, 512], f32, tag=f"pqa{j}", name=f"pqa{j}")
                           for j in (0, 1)]
                    pka = [pj.tile([128, 512], f32, tag=f"pka{j}", name=f"pka{j}")
                           for j in (0, 1)]
                    pb = [pj.tile([128, 512], f32, tag=f"pb{j}", name=f"pb{j}")
                          for j in (0, 1)]
                    for kc in range(KC):
                        st, sp = kc == 0, kc == KC - 1
                        for j in (0, 1):
                            nc.tensor.matmul(pqa[j][:], w_q[:, kc, 0:128],
                                             x_ch[kc][:, ss[j]], start=st, stop=sp)
                        for j in (0, 1):
                            nc.tensor.matmul(pka[j][:], w_k[:, kc, 0:128],
                                             x_ch[kc][:, ss[j]], start=st, stop=sp)
                        for j in (0, 1):
                            nc.tensor.matmul(pb[j][0:64, :], w_q[:, kc, 128:192],
                                             x_ch[kc][:, ss[j]], start=st, stop=sp,
                                             tile_position=(0, 0), skip_group_check=True)
                        for j in (0, 1):
                            nc.tensor.matmul(pb[j][64:128, :], w_k[:, kc, 128:192],
                                             x_ch[kc][:, ss[j]], start=st, stop=sp,
                                             tile_position=(0, 64), skip_group_check=True)
                    for j in (0, 1):
                        nc.scalar.activation(q01[:, ss[j]], pqa[j][:], Ident,
                                             bias=bq01[:])
                        nc.scalar.activation(k01[:, ss[j]], pka[j][:], Ident,
                                             bias=bk01[:])
                        nc.scalar.activation(q2[0:64, ss[j]], pb[j][0:64, :], Ident,
                                             bias=bq2[:])
                        nc.vector.tensor_scalar_add(k2[0:64, ss[j]], pb[j][64:128, :], bk2[:])

            # ---- Phase 1b: V projection into natural [t, d] layout ----
            with tc.tile_pool(name="pv", bufs=4, space=PSUM) as pvp:
                for tt in range(TT):
                    ts_ = slice(tt * 128, (tt + 1) * 128)
                    pvt = pvp.tile([128, HPC * DH], f32, tag="pvt")
                    nc.tensor.matmul(pvt[:], ones1[:], bv_sb[:], start=True, stop=False)
                    for kc in range(KC):
                        nc.tensor.matmul(
                            pvt[:], x_ch[kc][:, ts_], w_v[:, kc, :],
                            start=False, stop=kc == KC - 1,
                        )
                    nc.scalar.copy(
                        v_sb[:, tt, :, 0:64],
                        pvt[:].rearrange("p (h d) -> p h d", h=HPC),
                    )

            # duplicate h2's Q/K to partitions 64..127 for self-paired row tiling
            nc.sync.dma_start(q2[64:128, :], q2[0:64, :])
            nc.sync.dma_start(k2[64:128, :], k2[0:64, :])

            if ablate == "p1":
                return

            # ---- Phase 2: attention + output projection, per q tile ----
            # PSUM: spool d0,d1 ([128,1024] = 2 banks each) = 4 banks;
            # opool o0,o1,o2 = 3; mpool aux = 1 (bc broadcasts + Wo blocks).
            with (
                tc.tile_pool(name="spool", bufs=1, space=PSUM) as spool,
                tc.tile_pool(name="opool", bufs=1, space=PSUM) as opool,
                tc.tile_pool(name="mpool", bufs=1, space=PSUM) as mpool,
                tc.tile_pool(name="ppool", bufs=3) as ppool,
                tc.tile_pool(name="npool", bufs=2) as npool,
                tc.tile_pool(name="ypool", bufs=2) as ypool,
            ):
                deferred = []  # per-qtile tail work interleaved into next qtile
                pend = []      # PV matmuls, flushed one group behind (carried
                               # across q-tile boundaries to avoid a PE drain
                               # stall at each boundary)

                def flush(n):
                    while len(pend) > n:
                        out, lhsT, rhs, st_, sp_ = pend.pop(0)
                        nc.tensor.matmul(out, lhsT, rhs, start=st_,
                                         stop=sp_, skip_group_check=True)

                for qt in range(NT):
                    qs = slice(qt * 512, (qt + 1) * 512)

                    ol = [opool.tile([DH + 1, 512], f32, tag=f"o{i}", name=f"ol{i}")
                          for i in (0, 1)]
                    o2 = opool.tile([DH + 1, 512], f32, tag="o2", name="o2")

                    def norm(o_acc, out_ap):
                        # denom row -> SBUF (ACT), K=1 broadcast (PE, f32r),
                        # reciprocal (DVE), one multiply (DVE).
                        lrow = npool.tile([DH + 1, 512], f32r, tag="lr")
                        nc.scalar.copy(lrow[DH : DH + 1, :], o_acc[DH : DH + 1, :])
                        bct = mpool.tile([128, 512], f32, tag="aux", name="bct")
                        bc = bct[0 : DH + 1, :]
                        nc.tensor.matmul(bc, ones65[DH : DH + 1, :],
                                         lrow[DH : DH + 1, :], start=True, stop=True)
                        rc = npool.tile([DH + 1, 512], f32, tag="rc")
                        nc.vector.reciprocal_approx_fast(rc[:], bc)
                        nc.vector.tensor_mul(out_ap, o_acc[0:DH, :], rc[0:DH, :])

                    # 24 groups of 2 steps; PV batched one group behind.
                    for g in range(24):
                        for j in (0, 1):
                            step = 2 * g + j
                            dt_ = spool.tile([128, 1024], f32, tag=f"d{j}",
                                             name=f"d{j}")
                            if step < TT:
                                kt = step
                                ks = slice(kt * 128, (kt + 1) * 128)
                                nc.tensor.matmul(dt_[:, 0:512], k01[0:64, ks],
                                                 q01[0:64, qs], start=True, stop=True,
                                                 skip_group_check=True)
                                nc.tensor.matmul(dt_[:, 512:1024], k01[64:128, ks],
                                                 q01[64:128, qs], start=True, stop=True,
                                                 skip_group_check=True)
                            else:
                                p = step - TT
                                ka = slice(2 * p * 128, (2 * p + 1) * 128)
                                kb = slice((2 * p + 1) * 128, (2 * p + 2) * 128)
                                nc.tensor.matmul(dt_[:, 0:512], k2[0:64, ka],
                                                 q2[0:64, qs], start=True, stop=True,
                                                 skip_group_check=True)
                                nc.tensor.matmul(dt_[:, 512:1024], k2[64:128, kb],
                                                 q2[64:128, qs], start=True, stop=True,
                                                 skip_group_check=True)
                            pt = ppool.tile([128, 1024], bf16, tag=f"p{j}",
                                            name=f"p{j}")
                            if step % 2 == 0:
                                nc.scalar.activation(pt[:, 0:XSPL], dt_[:, 0:XSPL],
                                                     Exp, scale=0.125)
                                nc.vector.tensor_scalar(
                                    pt[:, XSPL:1024].bitcast(i16), dt_[:, XSPL:1024],
                                    SCH_A, SCH_B, Mult, Add)
                            else:
                                nc.vector.tensor_scalar(
                                    pt[:, 0:1024 - XSPL].bitcast(i16),
                                    dt_[:, 0:1024 - XSPL],
                                    SCH_A, SCH_B, Mult, Add)
                                nc.scalar.activation(pt[:, 1024 - XSPL:1024],
                                                     dt_[:, 1024 - XSPL:1024],
                                                     Exp, scale=0.125)
                            if step < TT:
                                kt = step
                                fi, la = kt == 0, kt == TT - 1
                                pend.append((ol[0][:], v_sb[:, kt, 0, 0:65],
                                             pt[:, 0:512], fi, la))
                                pend.append((ol[1][:], v_sb[:, kt, 1, 0:65],
                                             pt[:, 512:1024], fi, la))
                            else:
                                p = step - TT
                                pend.append((o2[:], v_sb[:, 2 * p, 2, 0:65],
                                             pt[:, 0:512], p == 0, False))
                                pend.append((o2[:], v_sb[:, 2 * p + 1, 2, 0:65],
                                             pt[:, 512:1024], False, p == TT // 2 - 1))
                        flush(6)
                        if deferred and 1 <= g <= 12 and g % 2 == 1:
                            deferred.pop(0)()
                        if g == 17:
                            # h0/h1 PV chains drained during group 16's flush;
                            # normalize them while the h2 groups continue.
                            norm(ol[0], on01[0:DH, qs])
                        if g == 19:
                            norm(ol[1], on01[DH:128, qs])

                    def defer_norm_o2(o2=o2, qs=qs):
                        norm(o2, on2[0:DH, qs])
                    deferred.append(defer_norm_o2)

                    def defer_wo(tt4, qt=qt):
                        t0 = qt * 512 + tt4 * 128
                        ts_ = slice(t0, t0 + 128)
                        ysb = ypool.tile([128, D], bf16, tag="ysb", name="ysb")
                        for m0, mw in ((0, 512), (512, 256)):
                            ms = slice(m0, m0 + mw)
                            yps = mpool.tile([128, 512], f32, tag="aux", name="yps")
                            nc.tensor.matmul(yps[:, 0:mw], on01[:, ts_], wo01_sb[:, ms],
                                             start=True, stop=False)
                            nc.tensor.matmul(yps[:, 0:mw], on2[:, ts_], wo2_sb[:, ms],
                                             start=False, stop=True)
                            nc.scalar.copy(ysb[:, ms], yps[:, 0:mw])
                        nc.sync.dma_start(y[ts_, :], ysb[:])

                    for tt4 in range(4):
                        deferred.append(lambda tt4=tt4: defer_wo(tt4))

                # drain the last qtile's PVs and tail
                flush(0)
                for f in deferred:
                    f()


def _build(iters=1):
    import concourse.bacc as bacc
    import concourse.tile as tile
    from concourse import mybir

    f32 = mybir.dt.float32
    bf16 = mybir.dt.bfloat16
    nc = bacc.Bacc("TRN2", target_bir_lowering=False, debug=False, name="mhsa")

    tens = (
        nc.dram_tensor("xT", [D, T], bf16, kind="ExternalInput"),
        nc.dram_tensor("wqT", [D, HPC * DH], bf16, kind="ExternalInput"),
        nc.dram_tensor("wkT", [D, HPC * DH], bf16, kind="ExternalInput"),
        nc.dram_tensor("wvT", [D, HPC * DH], bf16, kind="ExternalInput"),
        nc.dram_tensor("bq", [HPC * DH, 1], f32, kind="ExternalInput"),
        nc.dram_tensor("bk", [HPC * DH, 1], f32, kind="ExternalInput"),
        nc.dram_tensor("bv", [1, HPC * DH], bf16, kind="ExternalInput"),
        nc.dram_tensor("wo01", [128, D], bf16, kind="ExternalInput"),
        nc.dram_tensor("wo2", [DH, D], bf16, kind="ExternalInput"),
        nc.dram_tensor("onesb", [1, T], bf16, kind="ExternalInput"),
        nc.dram_tensor("ones65", [1, DH + 1], mybir.dt.float32r, kind="ExternalInput"),
        nc.dram_tensor("y", [T, D], bf16, kind="ExternalOutput"),
    )
    with tile.TileContext(nc) as tc:
        _trace(nc, tc, mybir, tens, iters)
    nc.finalize()
    return nc


def _prep_inputs(x, Wq, bq, Wk, bk, Wv, bv, Wo, bo):
    in_maps = []
    xTb = [np.ascontiguousarray(x[b].T).astype(BF16) for b in range(B)]
    for c in range(NCORES):
        b = c // 4
        h0 = (c % 4) * HPC
        cols = slice(h0 * DH, (h0 + HPC) * DH)
        woT = np.ascontiguousarray(Wo[:, cols].T)  # [192, 768]
        wo01 = np.ascontiguousarray(woT[0:128]).astype(BF16)
        wo2 = np.ascontiguousarray(woT[128:192]).astype(BF16)
        in_maps.append(
            {
                "xT": xTb[b],
                "wqT": np.ascontiguousarray(Wq[cols, :].T).astype(BF16),
                "wkT": np.ascontiguousarray(Wk[cols, :].T).astype(BF16),
                "wvT": np.ascontiguousarray(Wv[cols, :].T).astype(BF16),
                "bq": np.ascontiguousarray(bq[cols]).reshape(-1, 1).astype(np.float32),
                "bk": np.ascontiguousarray(bk[cols]).reshape(-1, 1).astype(np.float32),
                "bv": np.ascontiguousarray(bv[cols]).reshape(1, -1).astype(BF16),
                "wo01": wo01,
                "wo2": wo2,
                "onesb": np.ones((1, T), dtype=BF16),
                "ones65": np.ones((1, DH + 1), dtype=np.float32),
            }
        )
    return in_maps


def kernel(x, Wq, bq, Wk, bk, Wv, bv, Wo, bo):
    x = np.asarray(x, dtype=np.float32)
    Wq, bq = np.asarray(Wq, np.float32), np.asarray(bq, np.float32)
    Wk, bk = np.asarray(Wk, np.float32), np.asarray(bk, np.float32)
    Wv, bv = np.asarray(Wv, np.float32), np.asarray(bv, np.float32)
    Wo, bo = np.asarray(Wo, np.float32), np.asarray(bo, np.float32)

    from concourse.bass_utils import run_bass_kernel_spmd

    iters = int(os.environ.get("MHSA_ITERS", "1"))
    key = ("nc", iters, os.environ.get("MHSA_ABLATE", ""),
           os.environ.get("MHSA_X", ""))
    if key not in _CACHE:
        _CACHE[key] = _build(iters)
    nc = _CACHE[key]

    in_maps = _prep_inputs(x, Wq, bq, Wk, bk, Wv, bv, Wo, bo)
    trace = bool(os.environ.get("MHSA_TRACE"))
    ncores = int(os.environ.get("MHSA_NCORES", NCORES))
    res = run_bass_kernel_spmd(
        nc, in_maps[:ncores], core_ids=list(range(ncores)), trace=trace
    )
    if res.exec_time_ns is not None:
        print(f"HW exec time: {res.exec_time_ns} ns")
        _CACHE["exec_time_ns"] = res.exec_time_ns
        _CACHE["trace"] = res.instructions_and_trace

    out = np.zeros((B, T, D), dtype=np.float32)
    for c in range(ncores):
        out[c // 4] += res.results[c]["y"].astype(np.float32)
    out += bo[None, None, :]
    return out
